# revision 10
# baseline (speedup 1.0000x reference)
"""Trainium2 Bass kernel for nn_Decoder (VQ codebook decoder).

Pipeline (per batch b): gather codebook entries by index, scale, sum over
quantizers, per-group linear projection -> z [L, 1024]; ConvTranspose1d
(1024->1024, k=4, s=2, SAME) -> zu [2L, 1024]; Conv1d (1024->512, k=7, SAME)
-> x [B, 512, 2L].

Sharding: 8 cores = (batch b in 0..3) x (half of L). Each core computes
x[b, :, half*4096 : (half+1)*4096] independently (halo frames recomputed
locally; conv SAME zero-padding handled via data-driven masks).

Per-core decomposition (frames l, l0 = half*2048):
  zc frames [l0-3, l1+2) (2053): VQ gather via gpsimd ap_gather from a
    [128, 1025] table (partition = 16*q + r: r<8 codebook dim d, r==8 a
    carrier row of ones, col 1024 = 0 for padding indices), then one fp32
    matmul with W2 [128, 512] per group (scales*W_out rows + b_out in the
    carrier row) -> zc [1024, 2053]. Out-of-range frames gather the zero
    column so zc (including bias) is exactly 0 there.
  Upsample phase-split: ze[m] = W_up[:,:,0]^T zc[m-1] + W_up[:,:,2]^T zc[m]
    (+ b_up via a K=1 matmul against a validity row), zo[m] = W_up[:,:,1]^T
    zc[m] + W_up[:,:,3]^T zc[m+1] (+ b_up); zu[2m]=ze[m], zu[2m+1]=zo[m].
    ze frames [l0-1, l1+2), zo frames [l0-2, l1+1) -> HBM scratch.
  Head conv phase-split over t=2n / 2n+1, accumulating 7 taps x 8 ci tiles
    of [128,128]x[128,512] matmuls per output tile; + b_head on the drain;
    even/odd interleaved into [128, 1024] tiles and DMA'd out.
"""
import numpy as np
import ml_dtypes

import concourse.mybir as mybir
import concourse.tile as tile
from concourse import bacc
from concourse.bass_utils import run_bass_kernel_spmd

# structural constants (hardcoded per contest contract)
G, Q, C, D = 2, 8, 1024, 8
DIMS, DPG = 1024, 512
B, L = 4, 4096
HEAD_OUT = 512
LLOC = 2048          # frames per core
EFR = 2053           # zc frames [l0-3, l1+2)
NZ = 2051            # ze / zo frames
NIDX = 2064          # gather count (mult of 16, >= EFR)
NCH_B = [512, 512, 512, 512, 3]   # stage B chunk sizes (sum = NZ)
NCH_A = [512, 512, 512, 512, 5]   # stage A chunk sizes (sum = EFR)
NCH_C = 4            # stage C chunks of 512 output pairs

f32 = mybir.dt.float32
i16 = mybir.dt.int16

DT_CONV = "bf16"  # "bf16" | "f32r"


def _dt(dt_conv):
    if dt_conv == "bf16":
        return mybir.dt.bfloat16, ml_dtypes.bfloat16
    elif dt_conv == "f32r":
        return mybir.dt.float32r, np.float32
    raise ValueError(dt_conv)


# ---------------------------------------------------------------- host prep
def prep_shared(codebooks, scales, W_out, b_out, W_up, b_up, W_head, b_head,
                dt_conv):
    """Core-independent packed arrays."""
    _, npdt = _dt(dt_conv)
    cbT = np.zeros((G, 128, C + 1), np.float32)
    W2 = np.zeros((G, 128, DPG), np.float32)
    for g in range(G):
        for q in range(Q):
            cbT[g, 16 * q : 16 * q + 8, :C] = codebooks[g, q].T
            cbT[g, 16 * q + 8, :C] = 1.0
            W2[g, 16 * q : 16 * q + 8] = (
                scales[g, q][:, None] * W_out[g]
            )
        W2[g, 8] = b_out[g]  # bias carrier: q=0's ones-row only
    # Wup_pack[phase, ci, ci_t, tap, co] ; phase e taps (k=0, k=2), o (1, 3)
    wup = np.empty((2, 128, 8, 2, DIMS), np.float32)
    for ph, (ka, kb) in enumerate(((0, 2), (1, 3))):
        for ci_t in range(8):
            blk = W_up[ci_t * 128 : (ci_t + 1) * 128]  # [128, co, k]
            wup[ph, :, ci_t, 0] = blk[:, :, ka]
            wup[ph, :, ci_t, 1] = blk[:, :, kb]
    whead = np.empty((128, 8, 7, HEAD_OUT), np.float32)
    for ci_t in range(8):
        # W_head is OIH [co, ci, k] -> [ci, k, co]
        whead[:, ci_t] = W_head[:, ci_t * 128 : (ci_t + 1) * 128].transpose(1, 2, 0)
    bup = b_up.reshape(1, DIMS)
    bhead = b_head.reshape(4, 128).T  # [128, co_t]
    return dict(
        cbT=cbT.astype(np.float32),
        W2=W2.astype(np.float32),
        wup=wup.astype(npdt),
        whead=whead.astype(npdt),
        bup=bup.astype(npdt),
        bhead=bhead.astype(np.float32),
        ones1=np.ones((1, 128), npdt),
    )


def prep_core(indices, core, dt_conv):
    """Per-core index array + validity mask row."""
    _, npdt = _dt(dt_conv)
    b, half = core // 2, core % 2
    l0 = half * LLOC
    ls = np.arange(NIDX) + l0 - 3
    valid = (ls >= 0) & (ls < L)
    lc = np.clip(ls, 0, L - 1)
    idx16 = np.empty((G, 128, NIDX // 16), np.int16)
    for g in range(G):
        for q in range(Q):
            v = indices[b, g * Q + q, lc]
            stream = np.where(valid & (v >= 0), v, C).astype(np.int16)
            idx16[g, 16 * q : 16 * q + 16] = stream.reshape(NIDX // 16, 16).T
    # vmask col v <-> frame m = l0-2+v, v in [0, 2052)
    ms = np.arange(NZ + 1) + l0 - 2
    vmask = ((ms >= 0) & (ms < L)).astype(npdt).reshape(1, NZ + 1)
    return dict(idx=idx16, vmask=vmask)


# ---------------------------------------------------------------- bass build
def build_nc(dt_conv=DT_CONV, debug=False):
    dt, _ = _dt(dt_conv)
    nc = bacc.Bacc("TRN2", target_bir_lowering=False, debug=False)

    cbT_d = nc.dram_tensor("cbT", [G, 128, C + 1], f32, kind="ExternalInput")
    idx_d = nc.dram_tensor("idx", [G, 128, NIDX // 16], i16, kind="ExternalInput")
    W2_d = nc.dram_tensor("W2", [G, 128, DPG], f32, kind="ExternalInput")
    wup_d = nc.dram_tensor("wup", [2, 128, 8, 2, DIMS], dt, kind="ExternalInput")
    whead_d = nc.dram_tensor("whead", [128, 8, 7, HEAD_OUT], dt, kind="ExternalInput")
    bup_d = nc.dram_tensor("bup", [1, DIMS], dt, kind="ExternalInput")
    vmask_d = nc.dram_tensor("vmask", [1, NZ + 1], dt, kind="ExternalInput")
    ones_d = nc.dram_tensor("ones1", [1, 128], dt, kind="ExternalInput")
    bhead_d = nc.dram_tensor("bhead", [128, 4], f32, kind="ExternalInput")
    x_d = nc.dram_tensor("x", [HEAD_OUT, 2 * LLOC], f32, kind="ExternalOutput")
    if debug:
        Xdbg_d = nc.dram_tensor("Xdbg", [128, G * NIDX], f32, kind="ExternalOutput")
        zcdbg_d = nc.dram_tensor("zcdbg", [128, 8 * EFR], dt, kind="ExternalOutput")
        zedbg_d = nc.dram_tensor("zedbg", [DIMS, NZ], dt, kind="ExternalOutput")
        zodbg_d = nc.dram_tensor("zodbg", [DIMS, NZ], dt, kind="ExternalOutput")

    with tile.TileContext(nc) as tc:
        with tc.tile_pool(name="dram", bufs=1, space="DRAM") as dpool:
            zes_d = dpool.tile([DIMS, NZ], dt)
            zos_d = dpool.tile([DIMS, NZ], dt)
            # ---------------- pass 1: VQ -> zc -> ze/zo ----------------
            with (
                tc.tile_pool(name="p1c", bufs=1) as p1c,
                tc.tile_pool(name="p1w", bufs=1) as p1w,
                tc.tile_pool(name="p1s", bufs=4) as p1s,
                tc.tile_pool(name="psA", bufs=2, space="PSUM") as psA,
                tc.tile_pool(name="psB", bufs=4, space="PSUM") as psB,
            ):
                cbT = [p1c.tile([128, C + 1], f32, name=f"cbT{g}") for g in range(G)]
                idxs = [p1c.tile([128, NIDX // 16], i16, name=f"idxs{g}") for g in range(G)]
                W2t = p1c.tile([128, G * DPG], f32, name="W2t")
                X = [p1c.tile([128, NIDX], f32, name=f"X{g}") for g in range(G)]
                bup = p1c.tile([1, DIMS], dt, name="bup")
                vmask = p1c.tile([1, NZ + 1], dt, name="vmask")
                ones1 = p1c.tile([1, 128], dt, name="ones1")
                zc = p1c.tile([128, 8, EFR], dt, name="zc")

                for g in range(G):
                    nc.sync.dma_start(out=cbT[g][:], in_=cbT_d.ap()[g])
                    nc.sync.dma_start(out=idxs[g][:], in_=idx_d.ap()[g])
                    nc.sync.dma_start(
                        out=W2t[:, g * DPG : (g + 1) * DPG], in_=W2_d.ap()[g]
                    )
                nc.sync.dma_start(out=bup[:], in_=bup_d.ap())
                nc.sync.dma_start(out=vmask[:], in_=vmask_d.ap())
                nc.sync.dma_start(out=ones1[:], in_=ones_d.ap())

                # broadcast masks: mask[p, v] = vmask[v] for the first/last
                # stage-B chunk columns (kills conv-tap leakage into frames
                # outside [0, L) at the global sequence edges)
                masks = {}
                NT = NCH_B[-1]
                for key, lo, n in (
                    ("e0", 1, 512),
                    ("o0", 0, 512),
                    ("et", NZ - NT + 1, NT),
                    ("ot", NZ - NT, NT),
                ):
                    mp = psA.tile([128, n], f32, tag="psA")
                    nc.tensor.matmul(
                        mp[:], lhsT=ones1[:], rhs=vmask[:, lo : lo + n],
                        start=True, stop=True,
                    )
                    mt = p1c.tile([128, n], f32, name=f"mask_{key}")
                    nc.vector.tensor_copy(mt[:], mp[:])
                    masks[key] = mt

                # gather: X[g] [128, NIDX]
                for g in range(G):
                    nc.gpsimd.ap_gather(
                        X[g][:],
                        cbT[g][:],
                        idxs[g][:],
                        channels=128,
                        num_elems=C + 1,
                        d=1,
                        num_idxs=NIDX,
                    )

                # stage A: zc[ct] = W2[g][:, et*128:+128].T @ X[g]
                for g in range(G):
                    for et in range(4):
                        ct = g * 4 + et
                        c0 = 0
                        for N in NCH_A:
                            ps = psA.tile([128, N], f32, tag="psA")
                            nc.tensor.matmul(
                                ps[:],
                                lhsT=W2t[:, g * DPG + et * 128 : g * DPG + et * 128 + 128],
                                rhs=X[g][:, c0 : c0 + N],
                                start=True,
                                stop=True,
                            )
                            if ct % 2 == 0:
                                nc.vector.tensor_copy(zc[:, ct, c0 : c0 + N], ps[:])
                            else:
                                nc.scalar.activation(
                                    zc[:, ct, c0 : c0 + N], ps[:],
                                    mybir.ActivationFunctionType.Copy,
                                )
                            c0 += N

                # stage B sweeps: phase 0 (ze, taps k=0/2), phase 1 (zo, 1/3)
                for ph in range(2):
                    wph = p1w.tile([128, 8, 2, DIMS], dt, tag="wup", name=f"wup{ph}")
                    nc.sync.dma_start(out=wph[:], in_=wup_d.ap()[ph])
                    scr = zes_d if ph == 0 else zos_d
                    for ci, N in zip(
                        np.cumsum([0] + NCH_B[:-1]).tolist(), NCH_B
                    ):
                        for co_t in range(8):
                            ps = psB.tile([128, N], f32, tag="psB")
                            for ci_t in range(8):
                                for tap in range(2):
                                    # zc col offset: ze: m-1 -> i+1, m -> i+2
                                    #               zo: m   -> i+1, m+1 -> i+2
                                    off = ci + 1 + tap
                                    nc.tensor.matmul(
                                        ps[:],
                                        lhsT=wph[
                                            :, ci_t, tap,
                                            co_t * 128 : co_t * 128 + 128,
                                        ],
                                        rhs=zc[:, ci_t, off : off + N],
                                        start=(ci_t == 0 and tap == 0),
                                        stop=False,
                                    )
                            # b_up carrier: vmask col = i + (1 - ph)
                            nc.tensor.matmul(
                                ps[:],
                                lhsT=bup[:, co_t * 128 : co_t * 128 + 128],
                                rhs=vmask[:, ci + 1 - ph : ci + 1 - ph + N],
                                start=False,
                                stop=True,
                            )
                            st = p1s.tile([128, 512], dt, tag="p1s")
                            ckey = ("e" if ph == 0 else "o") + (
                                "0" if ci == 0 else ("t" if N == NCH_B[-1] and ci > 0 else "")
                            )
                            if ckey in masks:
                                nc.vector.tensor_tensor(
                                    out=st[:, :N], in0=ps[:], in1=masks[ckey][:],
                                    op=mybir.AluOpType.mult,
                                )
                            elif co_t % 2 == 0:
                                nc.vector.tensor_copy(st[:, :N], ps[:])
                            else:
                                nc.scalar.activation(
                                    st[:, :N], ps[:],
                                    mybir.ActivationFunctionType.Copy,
                                )
                            nc.scalar.dma_start(
                                out=scr[co_t * 128 : co_t * 128 + 128, ci : ci + N],
                                in_=st[:, :N],
                            )

                if debug:
                    for g in range(G):
                        nc.gpsimd.dma_start(
                            out=Xdbg_d.ap()[:, g * NIDX : (g + 1) * NIDX],
                            in_=X[g][:],
                        )
                    nc.gpsimd.dma_start(
                        out=zcdbg_d.ap(),
                        in_=zc[:].rearrange("p a b -> p (a b)"),
                    )
                    nc.gpsimd.dma_start(out=zedbg_d.ap(), in_=zes_d[:, :])
                    nc.gpsimd.dma_start(out=zodbg_d.ap(), in_=zos_d[:, :])
            # ---------------- pass 2: head conv ----------------
            with (
                tc.tile_pool(name="p2w", bufs=1) as p2w,
                tc.tile_pool(name="p2z", bufs=2) as p2z,
                tc.tile_pool(name="p2x", bufs=3) as p2x,
                tc.tile_pool(name="psC", bufs=4, space="PSUM") as psC,
            ):
                wh = p2w.tile([128, 8, 7, HEAD_OUT], dt, name="wh")
                nc.sync.dma_start(out=wh[:], in_=whead_d.ap())
                bh = p2w.tile([128, 4], f32, name="bh")
                nc.sync.dma_start(out=bh[:], in_=bhead_d.ap())

                # per-phase (tap k -> (src, col offset)) for x[2n]/x[2n+1]
                terms_even = [("e", 1, 0), ("e", 3, 1), ("e", 5, 2),
                              ("o", 0, 0), ("o", 2, 1), ("o", 4, 2), ("o", 6, 3)]
                terms_odd = [("e", 0, 0), ("e", 2, 1), ("e", 4, 2), ("e", 6, 3),
                             ("o", 1, 1), ("o", 3, 2), ("o", 5, 3)]

                for k in range(NCH_C):
                    i0 = 512 * k
                    ze = p2z.tile([128, 8, 515], dt, tag="ze", name=f"ze{k}")
                    zo = p2z.tile([128, 8, 515], dt, tag="zo", name=f"zo{k}")
                    for ci_t in range(8):
                        nc.sync.dma_start(
                            out=ze[:, ci_t],
                            in_=zes_d[ci_t * 128 : ci_t * 128 + 128, i0 : i0 + 515],
                        )
                        nc.sync.dma_start(
                            out=zo[:, ci_t],
                            in_=zos_d[ci_t * 128 : ci_t * 128 + 128, i0 : i0 + 515],
                        )
                    for co_t in range(4):
                        xt = p2x.tile([128, 1024], f32, tag="xt")
                        xv = xt[:].rearrange("p (n two) -> p two n", two=2)
                        for par, terms in enumerate((terms_even, terms_odd)):
                            ps = psC.tile([128, 512], f32, tag="psC")
                            first = True
                            for src, kk, coff in terms:
                                zt = ze if src == "e" else zo
                                for ci_t in range(8):
                                    nc.tensor.matmul(
                                        ps[:],
                                        lhsT=wh[
                                            :, ci_t, kk,
                                            co_t * 128 : co_t * 128 + 128,
                                        ],
                                        rhs=zt[:, ci_t, coff : coff + 512],
                                        start=first,
                                        stop=(src == terms[-1][0]
                                              and kk == terms[-1][1]
                                              and ci_t == 7),
                                    )
                                    first = False
                            if par == 0:
                                nc.scalar.activation(
                                    xv[:, par], ps[:],
                                    mybir.ActivationFunctionType.Identity,
                                    bias=bh[:, co_t : co_t + 1],
                                )
                            else:
                                nc.vector.tensor_scalar_add(
                                    xv[:, par], ps[:], bh[:, co_t : co_t + 1]
                                )
                        nc.scalar.dma_start(
                            out=x_d.ap()[
                                co_t * 128 : co_t * 128 + 128,
                                1024 * k : 1024 * k + 1024,
                            ],
                            in_=xt[:],
                        )
    nc.compile()
    return nc


# ---------------------------------------------------------------- entry
_CACHE = {}


def _get_nc(dt_conv):
    if dt_conv not in _CACHE:
        _CACHE[dt_conv] = build_nc(dt_conv)
    return _CACHE[dt_conv]


def make_in_maps(inputs, dt_conv=DT_CONV):
    shared = prep_shared(
        np.asarray(inputs["codebooks"], np.float32),
        np.asarray(inputs["scales"], np.float32),
        np.asarray(inputs["W_out"], np.float32),
        np.asarray(inputs["b_out"], np.float32),
        np.asarray(inputs["W_up"], np.float32),
        np.asarray(inputs["b_up"], np.float32),
        np.asarray(inputs["W_head"], np.float32),
        np.asarray(inputs["b_head"], np.float32),
        dt_conv,
    )
    indices = np.asarray(inputs["indices"])
    in_maps = []
    for core in range(8):
        m = dict(shared)
        pc = prep_core(indices, core, dt_conv)
        m["idx"] = pc["idx"]
        m["vmask"] = pc["vmask"]
        in_maps.append(m)
    return in_maps


def assemble(results):
    out = np.empty((B, HEAD_OUT, 2 * L), np.float32)
    for core in range(8):
        b, half = core // 2, core % 2
        out[b, :, half * 2 * LLOC : (half + 1) * 2 * LLOC] = results[core]["x"]
    return out


def kernel(**inputs):
    nc = _get_nc(DT_CONV)
    in_maps = make_in_maps(inputs, DT_CONV)
    res = run_bass_kernel_spmd(nc, in_maps, list(range(8)))
    return assemble(res.results)


# revision 14
# speedup vs baseline: 130.2459x; 130.2459x over previous
"""Trainium2 Bass kernel for nn_Decoder (VQ codebook decoder).

Pipeline (per batch b): gather codebook entries by index, scale, sum over
quantizers, per-group linear projection -> z [L, 1024]; ConvTranspose1d
(1024->1024, k=4, s=2, SAME) -> zu [2L, 1024]; Conv1d (1024->512, k=7, SAME)
-> x [B, 512, 2L].

Sharding: 8 cores = (batch b in 0..3) x (half of L). Each core computes
x[b, :, half*4096 : (half+1)*4096] independently (halo frames recomputed
locally; conv SAME zero-padding handled via data-driven masks).

Per-core decomposition (frames l, l0 = half*2048):
  zc frames [l0-3, l1+2) (2053): VQ gather via gpsimd ap_gather from a
    [128, 1025] table (partition = 16*q + r: r<8 codebook dim d, r==8 a
    carrier row of ones, col 1024 = 0 for padding indices), then one fp32
    matmul with W2 [128, 512] per group (scales*W_out rows + b_out in the
    carrier row) -> zc [1024, 2053]. Out-of-range frames gather the zero
    column so zc (including bias) is exactly 0 there.
  Upsample phase-split: ze[m] = W_up[:,:,0]^T zc[m-1] + W_up[:,:,2]^T zc[m]
    (+ b_up via a K=1 matmul against a validity row), zo[m] = W_up[:,:,1]^T
    zc[m] + W_up[:,:,3]^T zc[m+1] (+ b_up); zu[2m]=ze[m], zu[2m+1]=zo[m].
    ze frames [l0-1, l1+2), zo frames [l0-2, l1+1) -> HBM scratch.
  Head conv phase-split over t=2n / 2n+1, accumulating 7 taps x 8 ci tiles
    of [128,128]x[128,512] matmuls per output tile; + b_head on the drain;
    even/odd interleaved into [128, 1024] tiles and DMA'd out.
"""
import numpy as np
import ml_dtypes

import concourse.mybir as mybir
import concourse.tile as tile
from concourse import bacc
from concourse.bass_utils import run_bass_kernel_spmd

# structural constants (hardcoded per contest contract)
G, Q, C, D = 2, 8, 1024, 8
DIMS, DPG = 1024, 512
B, L = 4, 4096
HEAD_OUT = 512
LLOC = 2048          # frames per core
EFR = 2053           # zc frames [l0-3, l1+2)
NZ = 2051            # ze / zo frames
NIDX = 2064          # gather count (mult of 16, >= EFR)
NCH_B = [512, 512, 512, 512, 3]   # stage B chunk sizes (sum = NZ)
NCH_A = [512, 512, 512, 512, 5]   # stage A chunk sizes (sum = EFR)
NCH_C = 4            # stage C chunks of 512 output pairs

f32 = mybir.dt.float32
i16 = mybir.dt.int16

DT_CONV = "bf16"  # "bf16" | "f32r"


def _dt(dt_conv):
    if dt_conv == "bf16":
        return mybir.dt.bfloat16, ml_dtypes.bfloat16
    elif dt_conv == "f32r":
        return mybir.dt.float32r, np.float32
    raise ValueError(dt_conv)


# ---------------------------------------------------------------- host prep
def prep_shared(codebooks, scales, W_out, b_out, W_up, b_up, W_head, b_head,
                dt_conv):
    """Core-independent packed arrays."""
    _, npdt = _dt(dt_conv)
    cbT = np.zeros((G, 128, C + 1), np.float32)
    W2 = np.zeros((G, 128, DPG), np.float32)
    for g in range(G):
        for q in range(Q):
            cbT[g, 16 * q : 16 * q + 8, :C] = codebooks[g, q].T
            cbT[g, 16 * q + 8, :C] = 1.0
            W2[g, 16 * q : 16 * q + 8] = (
                scales[g, q][:, None] * W_out[g]
            )
        W2[g, 8] = b_out[g]  # bias carrier: q=0's ones-row only
    # Wup_pack[phase, ci, ci_t, tap, co] ; phase e taps (k=0, k=2), o (1, 3)
    wup = np.empty((2, 128, 8, 2, DIMS), np.float32)
    for ph, (ka, kb) in enumerate(((0, 2), (1, 3))):
        for ci_t in range(8):
            blk = W_up[ci_t * 128 : (ci_t + 1) * 128]  # [128, co, k]
            wup[ph, :, ci_t, 0] = blk[:, :, ka]
            wup[ph, :, ci_t, 1] = blk[:, :, kb]
    whead = np.empty((128, 8, 7, HEAD_OUT), np.float32)
    for ci_t in range(8):
        # W_head is OIH [co, ci, k] -> [ci, k, co]
        whead[:, ci_t] = W_head[:, ci_t * 128 : (ci_t + 1) * 128].transpose(1, 2, 0)
    bup = b_up.reshape(1, DIMS)
    bhead = b_head.reshape(4, 128).T  # [128, co_t]
    return dict(
        cbT=cbT.astype(np.float32),
        W2=W2.astype(np.float32),
        wup=wup.astype(npdt),
        whead=whead.astype(npdt),
        bup=bup.astype(npdt),
        bhead=bhead.astype(np.float32),
        ones1=np.ones((1, 128), npdt),
    )


def prep_core(indices, core, dt_conv):
    """Per-core index array + validity mask row."""
    _, npdt = _dt(dt_conv)
    b, half = core // 2, core % 2
    l0 = half * LLOC
    ls = np.arange(NIDX) + l0 - 3
    valid = (ls >= 0) & (ls < L)
    lc = np.clip(ls, 0, L - 1)
    idx16 = np.empty((G, 128, NIDX // 16), np.int16)
    for g in range(G):
        for q in range(Q):
            v = indices[b, g * Q + q, lc]
            stream = np.where(valid & (v >= 0), v, C).astype(np.int16)
            idx16[g, 16 * q : 16 * q + 16] = stream.reshape(NIDX // 16, 16).T
    # vmask col v <-> frame m = l0-2+v, v in [0, 2052)
    ms = np.arange(NZ + 1) + l0 - 2
    vmask = ((ms >= 0) & (ms < L)).astype(npdt).reshape(1, NZ + 1)
    return dict(idx=idx16, vmask=vmask)


# ---------------------------------------------------------------- bass build
def build_nc(dt_conv=DT_CONV, debug=False, reps=1, skip=()):
    dt, _ = _dt(dt_conv)
    nc = bacc.Bacc("TRN2", target_bir_lowering=False, debug=False)

    cbT_d = nc.dram_tensor("cbT", [G, 128, C + 1], f32, kind="ExternalInput")
    idx_d = nc.dram_tensor("idx", [G, 128, NIDX // 16], i16, kind="ExternalInput")
    W2_d = nc.dram_tensor("W2", [G, 128, DPG], f32, kind="ExternalInput")
    wup_d = nc.dram_tensor("wup", [2, 128, 8, 2, DIMS], dt, kind="ExternalInput")
    whead_d = nc.dram_tensor("whead", [128, 8, 7, HEAD_OUT], dt, kind="ExternalInput")
    bup_d = nc.dram_tensor("bup", [1, DIMS], dt, kind="ExternalInput")
    vmask_d = nc.dram_tensor("vmask", [1, NZ + 1], dt, kind="ExternalInput")
    ones_d = nc.dram_tensor("ones1", [1, 128], dt, kind="ExternalInput")
    bhead_d = nc.dram_tensor("bhead", [128, 4], f32, kind="ExternalInput")
    x_d = nc.dram_tensor("x", [HEAD_OUT, 2 * LLOC], f32, kind="ExternalOutput")
    if debug:
        Xdbg_d = nc.dram_tensor("Xdbg", [128, G * NIDX], f32, kind="ExternalOutput")
        zcdbg_d = nc.dram_tensor("zcdbg", [128, 8 * EFR], dt, kind="ExternalOutput")
        zedbg_d = nc.dram_tensor("zedbg", [DIMS, NZ], dt, kind="ExternalOutput")
        zodbg_d = nc.dram_tensor("zodbg", [DIMS, NZ], dt, kind="ExternalOutput")

    with tile.TileContext(nc) as tc:
      for rep in range(reps):
        with tc.tile_pool(name=f"dram{rep}", bufs=1, space="DRAM") as dpool:
            zes_d = dpool.tile([DIMS, NZ], dt)
            zos_d = dpool.tile([DIMS, NZ], dt)
            # ---------------- pass 1: VQ -> zc -> ze/zo ----------------
            with (
                tc.tile_pool(name=f"p1c{rep}", bufs=1) as p1c,
                tc.tile_pool(name=f"p1w{rep}", bufs=1) as p1w,
                tc.tile_pool(name=f"p1s{rep}", bufs=4) as p1s,
                tc.tile_pool(name=f"psA{rep}", bufs=2, space="PSUM") as psA,
                tc.tile_pool(name=f"psB{rep}", bufs=4, space="PSUM") as psB,
            ):
                cbT = [p1c.tile([128, C + 1], f32, name=f"cbT{g}") for g in range(G)]
                idxs = [p1c.tile([128, NIDX // 16], i16, name=f"idxs{g}") for g in range(G)]
                W2t = p1c.tile([128, G * DPG], f32, name="W2t")
                X = [p1c.tile([128, NIDX], f32, name=f"X{g}") for g in range(G)]
                bup = p1c.tile([1, DIMS], dt, name="bup")
                vmask = p1c.tile([1, NZ + 1], dt, name="vmask")
                ones1 = p1c.tile([1, 128], dt, name="ones1")
                zc = p1c.tile([128, 8, EFR], dt, name="zc")

                for g in range(G):
                    nc.sync.dma_start(out=cbT[g][:], in_=cbT_d.ap()[g])
                    nc.sync.dma_start(out=idxs[g][:], in_=idx_d.ap()[g])
                    nc.sync.dma_start(
                        out=W2t[:, g * DPG : (g + 1) * DPG], in_=W2_d.ap()[g]
                    )
                nc.sync.dma_start(out=bup[:], in_=bup_d.ap())
                nc.sync.dma_start(out=vmask[:], in_=vmask_d.ap())
                nc.sync.dma_start(out=ones1[:], in_=ones_d.ap())

                # broadcast masks: mask[p, v] = vmask[v] for the first/last
                # stage-B chunk columns (kills conv-tap leakage into frames
                # outside [0, L) at the global sequence edges)
                masks = {}
                NT = NCH_B[-1]
                for key, lo, n in (
                    ("e0", 1, 512),
                    ("o0", 0, 512),
                    ("et", NZ - NT + 1, NT),
                    ("ot", NZ - NT, NT),
                ):
                    mp = psA.tile([128, n], f32, tag="psA")
                    nc.tensor.matmul(
                        mp[:], lhsT=ones1[:], rhs=vmask[:, lo : lo + n],
                        start=True, stop=True,
                    )
                    mt = p1c.tile([128, n], f32, name=f"mask_{key}")
                    nc.vector.tensor_copy(mt[:], mp[:])
                    masks[key] = mt

                # gather: X[g] [128, NIDX]
                for g in range(G):
                    if "gather" in skip:
                        nc.gpsimd.memset(X[g][:], 0.25)
                        continue
                    nc.gpsimd.ap_gather(
                        X[g][:],
                        cbT[g][:],
                        idxs[g][:],
                        channels=128,
                        num_elems=C + 1,
                        d=1,
                        num_idxs=NIDX,
                    )

                # stage A: zc[ct] = W2[g][:, et*128:+128].T @ X[g]
                for g in range(G):
                    for et in range(4):
                        ct = g * 4 + et
                        c0 = 0
                        for N in NCH_A:
                            ps = psA.tile([128, N], f32, tag="psA")
                            nc.tensor.matmul(
                                ps[:],
                                lhsT=W2t[:, g * DPG + et * 128 : g * DPG + et * 128 + 128],
                                rhs=X[g][:, c0 : c0 + N],
                                start=True,
                                stop=True,
                            )
                            nc.vector.tensor_copy(zc[:, ct, c0 : c0 + N], ps[:])
                            c0 += N

                # stage B sweeps: phase 0 (ze, taps k=0/2), phase 1 (zo, 1/3)
                for ph in range(2):
                    wph = p1w.tile([128, 8, 2, DIMS], dt, tag="wup", name=f"wup{ph}")
                    nc.sync.dma_start(out=wph[:], in_=wup_d.ap()[ph])
                    scr = zes_d if ph == 0 else zos_d
                    for ci, N in zip(
                        np.cumsum([0] + NCH_B[:-1]).tolist(), NCH_B
                    ):
                        for co_t in range(8):
                            ps = psB.tile([128, N], f32, tag="psB")
                            for ci_t in range(8 if "Bmm" not in skip else 0):
                                for tap in range(2):
                                    # zc col offset: ze: m-1 -> i+1, m -> i+2
                                    #               zo: m   -> i+1, m+1 -> i+2
                                    off = ci + 1 + tap
                                    nc.tensor.matmul(
                                        ps[:],
                                        lhsT=wph[
                                            :, ci_t, tap,
                                            co_t * 128 : co_t * 128 + 128,
                                        ],
                                        rhs=zc[:, ci_t, off : off + N],
                                        start=(ci_t == 0 and tap == 0),
                                        stop=False,
                                    )
                            # b_up carrier: vmask col = i + (1 - ph)
                            nc.tensor.matmul(
                                ps[:],
                                lhsT=bup[:, co_t * 128 : co_t * 128 + 128],
                                rhs=vmask[:, ci + 1 - ph : ci + 1 - ph + N],
                                start=("Bmm" in skip),
                                stop=True,
                            )
                            st = p1s.tile([128, 512], dt, tag="p1s")
                            if "Bdrain" in skip:
                                nc.vector.tensor_copy(st[:, :1], ps[:, :1])
                                continue
                            ckey = ("e" if ph == 0 else "o") + (
                                "0" if ci == 0 else ("t" if N == NCH_B[-1] and ci > 0 else "")
                            )
                            if ckey in masks:
                                nc.vector.tensor_tensor(
                                    out=st[:, :N], in0=ps[:], in1=masks[ckey][:],
                                    op=mybir.AluOpType.mult,
                                )
                            else:
                                nc.vector.tensor_copy(st[:, :N], ps[:])
                            if "Bstore" not in skip:
                                nc.scalar.dma_start(
                                    out=scr[co_t * 128 : co_t * 128 + 128, ci : ci + N],
                                    in_=st[:, :N],
                                )

                if debug:
                    for g in range(G):
                        nc.gpsimd.dma_start(
                            out=Xdbg_d.ap()[:, g * NIDX : (g + 1) * NIDX],
                            in_=X[g][:],
                        )
                    nc.gpsimd.dma_start(
                        out=zcdbg_d.ap(),
                        in_=zc[:].rearrange("p a b -> p (a b)"),
                    )
                    nc.gpsimd.dma_start(out=zedbg_d.ap(), in_=zes_d[:, :])
                    nc.gpsimd.dma_start(out=zodbg_d.ap(), in_=zos_d[:, :])
            # ---------------- pass 2: head conv ----------------
            with (
                tc.tile_pool(name=f"p2w{rep}", bufs=1) as p2w,
                tc.tile_pool(name=f"p2z{rep}", bufs=2) as p2z,
                tc.tile_pool(name=f"p2x{rep}", bufs=3) as p2x,
                tc.tile_pool(name=f"psC{rep}", bufs=4, space="PSUM") as psC,
            ):
                wh = p2w.tile([128, 8, 7, HEAD_OUT], dt, name="wh")
                nc.sync.dma_start(out=wh[:], in_=whead_d.ap())
                bh = p2w.tile([128, 4], f32, name="bh")
                nc.sync.dma_start(out=bh[:], in_=bhead_d.ap())

                # per-phase (tap k -> (src, col offset)) for x[2n]/x[2n+1]
                terms_even = [("e", 1, 0), ("e", 3, 1), ("e", 5, 2),
                              ("o", 0, 0), ("o", 2, 1), ("o", 4, 2), ("o", 6, 3)]
                terms_odd = [("e", 0, 0), ("e", 2, 1), ("e", 4, 2), ("e", 6, 3),
                             ("o", 1, 1), ("o", 3, 2), ("o", 5, 3)]

                for k in range(NCH_C):
                    i0 = 512 * k
                    ze = p2z.tile([128, 8, 515], dt, tag="ze", name=f"ze{k}")
                    zo = p2z.tile([128, 8, 515], dt, tag="zo", name=f"zo{k}")
                    if "Cload" in skip:
                        nc.vector.memset(ze[:], 0.25)
                        nc.vector.memset(zo[:], 0.25)
                    for ci_t in range(8 if "Cload" not in skip else 0):
                        nc.sync.dma_start(
                            out=ze[:, ci_t],
                            in_=zes_d[ci_t * 128 : ci_t * 128 + 128, i0 : i0 + 515],
                        )
                        nc.sync.dma_start(
                            out=zo[:, ci_t],
                            in_=zos_d[ci_t * 128 : ci_t * 128 + 128, i0 : i0 + 515],
                        )
                    for co_t in range(4):
                        xt = p2x.tile([128, 1024], f32, tag="xt")
                        xv = xt[:].rearrange("p (n two) -> p two n", two=2)
                        for par, terms in enumerate((terms_even, terms_odd)):
                            ps = psC.tile([128, 512], f32, tag="psC")
                            first = True
                            for src, kk, coff in (terms if "Cmm" not in skip else terms[-1:]):
                                zt = ze if src == "e" else zo
                                for ci_t in (range(8) if "Cmm" not in skip else [7]):
                                    nc.tensor.matmul(
                                        ps[:],
                                        lhsT=wh[
                                            :, ci_t, kk,
                                            co_t * 128 : co_t * 128 + 128,
                                        ],
                                        rhs=zt[:, ci_t, coff : coff + 512],
                                        start=first,
                                        stop=(src == terms[-1][0]
                                              and kk == terms[-1][1]
                                              and ci_t == 7),
                                    )
                                    first = False
                            if "Cdrain" in skip:
                                nc.vector.tensor_copy(xt[:, par : par + 1], ps[:, :1])
                            else:
                                nc.vector.tensor_scalar_add(
                                    xv[:, par], ps[:], bh[:, co_t : co_t + 1]
                                )
                        if "Cdrain" in skip:
                            nc.scalar.dma_start(
                                out=x_d.ap()[
                                    co_t * 128 : co_t * 128 + 128,
                                    1024 * k : 1024 * k + 2,
                                ],
                                in_=xt[:, :2],
                            )
                        else:
                            nc.scalar.dma_start(
                                out=x_d.ap()[
                                    co_t * 128 : co_t * 128 + 128,
                                    1024 * k : 1024 * k + 1024,
                                ],
                                in_=xt[:],
                            )
    nc.compile()
    return nc


# ---------------------------------------------------------------- entry
_CACHE = {}


def _get_nc(dt_conv):
    if dt_conv not in _CACHE:
        _CACHE[dt_conv] = build_nc(dt_conv)
    return _CACHE[dt_conv]


def make_in_maps(inputs, dt_conv=DT_CONV):
    shared = prep_shared(
        np.asarray(inputs["codebooks"], np.float32),
        np.asarray(inputs["scales"], np.float32),
        np.asarray(inputs["W_out"], np.float32),
        np.asarray(inputs["b_out"], np.float32),
        np.asarray(inputs["W_up"], np.float32),
        np.asarray(inputs["b_up"], np.float32),
        np.asarray(inputs["W_head"], np.float32),
        np.asarray(inputs["b_head"], np.float32),
        dt_conv,
    )
    indices = np.asarray(inputs["indices"])
    in_maps = []
    for core in range(8):
        m = dict(shared)
        pc = prep_core(indices, core, dt_conv)
        m["idx"] = pc["idx"]
        m["vmask"] = pc["vmask"]
        in_maps.append(m)
    return in_maps


def assemble(results):
    out = np.empty((B, HEAD_OUT, 2 * L), np.float32)
    for core in range(8):
        b, half = core // 2, core % 2
        out[b, :, half * 2 * LLOC : (half + 1) * 2 * LLOC] = results[core]["x"]
    return out


def kernel(**inputs):
    nc = _get_nc(DT_CONV)
    in_maps = make_in_maps(inputs, DT_CONV)
    res = run_bass_kernel_spmd(nc, in_maps, list(range(8)))
    return assemble(res.results)
